# revision 41
# baseline (speedup 1.0000x reference)
"""Causal single-head attention (B=4, S=2048, D=1024, fp32) on 8 TRN2 cores.

Reference computation (per batch b):
    scores = (x @ qk) @ x.T / sqrt(D)   causal-masked, softmax over keys
    out    = softmax(scores) @ x @ ov

Sharding: 2 cores per batch. Each core owns 8 of the 16 128-row query
blocks, snake-assigned ({4k, 4k+3} vs {4k+1, 4k+2}) so both halves see an
identical causal work profile -> one SPMD program, per-core data only.

Per-core pipeline (all matmuls/transposes in float32r = full-rate PE,
PSUM fp32 accumulation):
  xS = x_rows.T (PE transposes, one 512-half at a time); qT = qk.T @ xS
  xT = x_full.T (PE transposes)
  per query block i: score strips = qT_i.T @ xT; additive causal mask
    built on-chip from iota vs a per-core threshold input; exp on ACT
    (accum_out gives row-sums for free); PE-transpose probs into attnT
  per 512-col strip of local rows: PT = sum_t x[t,:]^T attnT[t,:]
    (lhsT = natural x tiles streamed from DRAM, 4-chunk PSUM groups
    folded into an fp32 SBUF accumulator)
  out block = (PT_i).T @ ov, scaled by 1/rowsum during the ACT PSUM
    evacuation; rows written back compact, host re-scatters.
"""

import numpy as np

import concourse.bacc as bacc
import concourse.mybir as mybir
import concourse.tile as tile
from concourse.bass_interp import get_hw_module
from concourse.bass_utils import run_bass_kernel_spmd
from concourse.masks import make_identity

B, S, D = 4, 2048, 1024
NB = S // 128          # 16 row blocks per batch
NBL = NB // 2          # 8 row blocks per core
N_CORES = 8
SCALE = float(np.sqrt(D))
NEG = -1.0e30

# local block -> global block, per half (snake: exactly balanced causal work)
HALF_BLOCKS = [
    [0, 3, 4, 7, 8, 11, 12, 15],
    [1, 2, 5, 6, 9, 10, 13, 14],
]
# 512-wide score strips per local block (same for both halves)
CI = [1, 1, 2, 2, 3, 3, 4, 4]
# t-chunks per PT block-pair (local blocks 2p, 2p+1 share c -> exact extents,
# no zero padding needed in attnT)
E_PAIR = [4, 8, 12, 16]

F32 = mybir.dt.float32
F32R = mybir.dt.float32r


def _emit(nc, tc, x_full, x_rows, qk_in, ov_in, masks_in, y_out, ctx):
    f32r = lambda ap: ap.bitcast(F32R)
    DC = D // 128  # 8

    const = ctx.enter_context(tc.tile_pool(name="const", bufs=1))
    psA = ctx.enter_context(tc.tile_pool(name="psA", bufs=3, space="PSUM"))
    psT = ctx.enter_context(tc.tile_pool(name="psT", bufs=3, space="PSUM"))
    psP = ctx.enter_context(tc.tile_pool(name="psP", bufs=2, space="PSUM"))

    ident = const.tile([128, 128], F32, name="ident")
    make_identity(nc, ident)
    ident_r = const.tile([128, 128], F32R, name="ident_r")
    nc.vector.tensor_copy(ident_r, ident)
    # iota 0..511 along free dim; causal mask for block i's last strip is
    # (iota > thresh[:, i]) * NEG with thresh a per-core input
    iota_t = const.tile([128, 512], F32, name="iota_t")
    nc.gpsimd.iota(iota_t, pattern=[[1, 512]], base=0, channel_multiplier=0,
                   allow_small_or_imprecise_dtypes=True)
    thresh_sb = const.tile([128, NBL], F32, name="thresh_sb")
    nc.sync.dma_start(out=thresh_sb, in_=masks_in)
    recips = const.tile([128, NBL], F32, name="recips")

    at_pool = ctx.enter_context(tc.tile_pool(name="attnT", bufs=1))
    attnT = [
        at_pool.tile([128, E_PAIR[pi], 256], F32R, name=f"attnT{pi}")
        for pi in range(4)
    ]

    # entered before qT/xT so its slots live below them in the pool stack:
    # the phase-3 x reload DMAs can then prefetch during phase 2 instead of
    # waiting for the xT/qT releases
    xn_p = ctx.enter_context(tc.tile_pool(name="xn_p", bufs=6))

    qT_pool = tc.tile_pool(name="qT", bufs=1)
    qTp = qT_pool.__enter__()
    qT = qTp.tile([128, DC, 1024], F32R, name="qT")

    # ---- phase 1a: xS = x_rows.T ; qT = qk.T @ xS  (one 512-row half of
    # x_rows at a time; the half-sized xS scratch is reused, the WAR dep
    # keeps PE busy with the qT matmuls in between) ----
    with tc.tile_pool(name="p1a", bufs=1) as p1a, \
         tc.tile_pool(name="xin_a", bufs=4) as xin_a:
        xS = p1a.tile([128, DC, 512], F32R, name="xS")
        qk_sb = p1a.tile([128, DC, 1024], F32R, name="qk_sb")
        qk_src = f32r(qk_in.rearrange("(c p) e -> p c e", p=128))
        xts_all = []
        for rbg in range(2):
            group = []
            for j in range(4):
                xt = xin_a.tile([128, 1024], F32R, name="xta", tag="xta")
                nc.sync.dma_start(
                    out=xt,
                    in_=f32r(x_rows[(rbg * 4 + j) * 128:(rbg * 4 + j + 1) * 128, :]))
                group.append(xt)
            xts_all.append(group)
            if rbg == 0:
                # qk chunks between the two x_rows groups in the DMA queue
                for dc in range(DC):
                    nc.sync.dma_start(
                        out=qk_sb[:, dc, :], in_=qk_src[:, dc, :])
        for rbg in range(2):
            xts = xts_all[rbg]
            for dc in range(DC):
                ps = psT.tile([128, 512], F32, name="ps_tr", tag="psT")
                for j in range(4):
                    nc.tensor.transpose(
                        ps[:, j * 128:(j + 1) * 128].bitcast(F32R),
                        xts[j][:, dc * 128:(dc + 1) * 128], ident_r)
                nc.vector.tensor_copy(xS[:, dc, :], ps)
            for ec in range(DC):
                ps = psA.tile([128, 512], F32, name="ps_qt", tag="psA")
                for dc in range(DC):
                    nc.tensor.matmul(
                        ps, qk_sb[:, dc, ec * 128:(ec + 1) * 128],
                        xS[:, dc, :],
                        start=(dc == 0), stop=(dc == DC - 1))
                nc.vector.tensor_copy(
                    qT[:, ec, rbg * 512:(rbg + 1) * 512], ps)

    # ---- phase 1b: xT = x_full.T ----
    xT_pool = tc.tile_pool(name="xT", bufs=1)
    xTp = xT_pool.__enter__()
    xT = xTp.tile([128, DC, S], F32R, name="xT")
    with tc.tile_pool(name="xin_b", bufs=8) as xin_b:
        for tbg in range(4):          # groups of 4 key blocks
            xts = []
            for j in range(4):
                xt = xin_b.tile([128, 1024], F32R, name="xtb", tag="xtb")
                nc.sync.dma_start(
                    out=xt,
                    in_=f32r(x_full[(tbg * 4 + j) * 128:(tbg * 4 + j + 1) * 128, :]))
                xts.append(xt)
            for dc in range(DC):
                ps = psT.tile([128, 512], F32, name="ps_tr", tag="psT")
                for j in range(4):
                    nc.tensor.transpose(
                        ps[:, j * 128:(j + 1) * 128].bitcast(F32R),
                        xts[j][:, dc * 128:(dc + 1) * 128], ident_r)
                nc.vector.tensor_copy(
                    xT[:, dc, tbg * 512:(tbg + 1) * 512], ps)

    # ---- phase 2: scores -> exp -> attnT, per local block ----
    with tc.tile_pool(name="p2", bufs=4) as p2, \
         tc.tile_pool(name="p2s", bufs=4) as p2s:
        for i in range(NBL):
            c = CI[i]
            pi = i // 2
            col = i % 2
            rs = p2s.tile([128, 4], F32, name="rs", tag="rs")
            for st in range(c):
                ps = psA.tile([128, 512], F32, name="ps_sc", tag="psA")
                for ec in range(DC):
                    nc.tensor.matmul(
                        ps, qT[:, ec, i * 128:(i + 1) * 128],
                        xT[:, ec, st * 512:(st + 1) * 512],
                        start=(ec == 0), stop=(ec == DC - 1))
                if st == c - 1:
                    mask = p2s.tile([128, 512], F32, name="mask", tag="mask",
                                    bufs=2)
                    nc.vector.tensor_scalar(
                        out=mask, in0=iota_t,
                        scalar1=thresh_sb[:, i:i + 1], scalar2=NEG,
                        op0=mybir.AluOpType.is_gt, op1=mybir.AluOpType.mult)
                    nc.vector.tensor_add(ps, ps, mask)
                p_st = p2.tile([128, 512], F32R, name="p_st", tag="p_st")
                nc.scalar.activation(
                    p_st, ps,
                    mybir.ActivationFunctionType.Exp,
                    scale=1.0 / SCALE, accum_out=rs[:, st:st + 1])
                pst2 = psT.tile([128, 512], F32, name="ps_at", tag="psT")
                for j in range(4):
                    nc.tensor.transpose(
                        pst2[:, j * 128:(j + 1) * 128].bitcast(F32R),
                        p_st[:, j * 128:(j + 1) * 128], ident_r)
                nc.vector.tensor_copy(
                    attnT[pi][:, st * 4:st * 4 + 4,
                              col * 128:(col + 1) * 128],
                    pst2.rearrange("p (a b) -> p a b", a=4))
            rsum = p2s.tile([128, 1], F32, name="rsum", tag="rsum")
            nc.vector.reduce_sum(rsum, rs[:, 0:c], axis=mybir.AxisListType.X)
            nc.vector.reciprocal(recips[:, i:i + 1], rsum)

    xT_pool.__exit__(None, None, None)
    qT_pool.__exit__(None, None, None)

    # ---- phase 3: PT = (attn @ x).T per block pair; out = PT.T @ ov ----
    # Each key chunk tc is loaded once; at group tcg it feeds every pair
    # whose causal extent reaches past 4*tcg (N=256 matmuls, exact extents).
    # Pair pi's accumulation completes at tcg == pi; its two out blocks are
    # emitted one group later so the paced ov chunks have arrived.
    with tc.tile_pool(name="p3", bufs=1) as p3, \
         tc.tile_pool(name="y_p", bufs=2) as y_p:
        ov_sb = p3.tile([128, DC, 1024], F32R, name="ov_sb")
        PT = p3.tile([128, DC, 1024], F32R, name="PT")
        # fp32 pair-scratch accumulators: PSUM can't hold 8 concurrent
        # d-chunk accumulators, so accumulate groups of 4 t-chunks in PSUM
        # and fold into SBUF
        PT32 = [p3.tile([128, DC, 256], F32, name=f"PT32_{pi}")
                for pi in range(4)]
        ov_dc = 0  # ov is loaded per-d-chunk, interleaved with xn groups
        ov_src = f32r(ov_in.rearrange("(c p) e -> p c e", p=128))

        def emit_out_pair(pi):
            for dc in range(DC):
                nc.vector.tensor_copy(
                    PT[:, dc, pi * 256:(pi + 1) * 256], PT32[pi][:, dc, :])
            for bi in range(2):
                i = 2 * pi + bi
                y_sb = y_p.tile([128, 1024], F32, name="y_sb", tag="y_sb")
                for es in range(2):
                    ps = psA.tile([128, 512], F32, name="ps_o", tag="psA")
                    for dc in range(DC):
                        nc.tensor.matmul(
                            ps, PT[:, dc, i * 128:(i + 1) * 128],
                            ov_sb[:, dc, es * 512:(es + 1) * 512],
                            start=(dc == 0), stop=(dc == DC - 1))
                    nc.scalar.activation(
                        y_sb[:, es * 512:(es + 1) * 512], ps,
                        mybir.ActivationFunctionType.Copy,
                        scale=recips[:, i:i + 1])
                nc.sync.dma_start(
                    out=y_out[i * 128:(i + 1) * 128, :], in_=y_sb)

        for tcg in range(4):
            xns = []
            for j in range(4):
                tc_idx = tcg * 4 + j
                xn = xn_p.tile([128, 1024], F32R, name="xn", tag="xn")
                nc.sync.dma_start(
                    out=xn,
                    in_=f32r(x_full[tc_idx * 128:(tc_idx + 1) * 128, :]))
                xns.append(xn)
            while ov_dc < min(DC, 4 * (tcg + 1)):
                nc.sync.dma_start(
                    out=ov_sb[:, ov_dc, :], in_=ov_src[:, ov_dc, :])
                ov_dc += 1
            for pi in range(4):
                if E_PAIR[pi] <= 4 * tcg:
                    continue
                for dc in range(DC):
                    ps = psP.tile([128, 256], F32, name="ps_pt", tag="psP")
                    for j in range(4):
                        nc.tensor.matmul(
                            ps, xns[j][:, dc * 128:(dc + 1) * 128],
                            attnT[pi][:, tcg * 4 + j, :],
                            start=(j == 0), stop=(j == 3))
                    if tcg == 0:
                        nc.vector.tensor_copy(PT32[pi][:, dc, :], ps)
                    else:
                        nc.vector.tensor_add(
                            PT32[pi][:, dc, :], PT32[pi][:, dc, :], ps)
            if tcg >= 1:
                emit_out_pair(tcg - 1)  # pair tcg-1 completed last group
        emit_out_pair(3)


_BUILT = {}


def _build(n_reps=1, timing=False):
    """timing=True builds a variant whose big tensors are Internal DRAM
    (garbage data, tiny external IO) so per-call transfer overhead over the
    axon tunnel doesn't swamp wall-clock differencing."""
    key = (n_reps, timing)
    if key in _BUILT:
        return _BUILT[key]
    from contextlib import ExitStack

    nc = bacc.Bacc(
        "TRN2", target_bir_lowering=False, debug=False,
        enable_asserts=False, num_devices=N_CORES)
    big = dict(kind="Internal") if timing else {}
    x_full = nc.dram_tensor("x_full", [S, D], F32,
                            **(big or dict(kind="ExternalInput"))).ap()
    x_rows = nc.dram_tensor("x_rows", [S // 2, D], F32,
                            **(big or dict(kind="ExternalInput"))).ap()
    qk_in = nc.dram_tensor("qk", [D, D], F32,
                           **(big or dict(kind="ExternalInput"))).ap()
    ov_in = nc.dram_tensor("ov", [D, D], F32,
                           **(big or dict(kind="ExternalInput"))).ap()
    masks_in = nc.dram_tensor(
        "thresh", [128, NBL], F32, kind="ExternalInput").ap()
    y_out = nc.dram_tensor("y", [S // 2, D], F32,
                           **(big or dict(kind="ExternalOutput"))).ap()
    dummy_out = None
    if timing:
        dummy_out = nc.dram_tensor(
            "dummy_y", [128, 128], F32, kind="ExternalOutput").ap()

    with tile.TileContext(nc) as tc:
        if timing and n_reps > 1:
            with tc.For_i(0, n_reps, 1):
                with ExitStack() as ctx:
                    _emit(nc, tc, x_full, x_rows, qk_in, ov_in, masks_in,
                          y_out, ctx)
        else:
            for _ in range(n_reps):
                with ExitStack() as ctx:
                    _emit(nc, tc, x_full, x_rows, qk_in, ov_in, masks_in,
                          y_out, ctx)
        if timing:
            with tc.tile_pool(name="dummy_p", bufs=1) as dp:
                dt_ = dp.tile([128, 128], F32, name="dummy_sb")
                nc.sync.dma_start(out=dt_, in_=y_out[0:128, 0:128])
                nc.sync.dma_start(out=dummy_out, in_=dt_)
    nc.compile()
    nc.m = get_hw_module(nc.m)
    _BUILT[key] = nc
    return nc


def host_thresh():
    """thresh[r, i] such that last-strip column tcol is causally valid for
    row r of local block i iff tcol <= thresh[r, i]."""
    th = np.zeros((2, 128, NBL), np.float32)
    for half in range(2):
        for i, g in enumerate(HALF_BLOCKS[half]):
            th[half, :, i] = 128 * g + np.arange(128) - 512 * (CI[i] - 1)
    return th


def make_in_maps(input_data, qk, ov):
    x = np.ascontiguousarray(np.asarray(input_data, dtype=np.float32))
    qk = np.ascontiguousarray(np.asarray(qk, dtype=np.float32))
    ov = np.ascontiguousarray(np.asarray(ov, dtype=np.float32))
    th = host_thresh()
    in_maps = []
    for c in range(N_CORES):
        b, half = c // 2, c % 2
        rows = np.concatenate(
            [x[b, 128 * g:128 * (g + 1), :] for g in HALF_BLOCKS[half]], axis=0)
        in_maps.append({
            "x_full": x[b],
            "x_rows": np.ascontiguousarray(rows),
            "qk": qk,
            "ov": ov,
            "thresh": np.ascontiguousarray(th[half]),
        })
    return in_maps


def assemble(results):
    out = np.empty((B, S, D), np.float32)
    for c in range(N_CORES):
        b, half = c // 2, c % 2
        y = results[c]["y"]
        for i, g in enumerate(HALF_BLOCKS[half]):
            out[b, 128 * g:128 * (g + 1), :] = y[128 * i:128 * (i + 1), :]
    return out


def kernel(input_data, qk, ov):
    nc = _build()
    in_maps = make_in_maps(input_data, qk, ov)
    res = run_bass_kernel_spmd(nc, in_maps, core_ids=list(range(N_CORES)))
    return assemble(res.results)
